# revision 99
# baseline (speedup 1.0000x reference)
"""MLA-style causal self-attention on 8 Trainium2 NeuronCores.

v24: tensor-parallel over heads (2/core). The kernel is PE-cycle-bound
at the GPIO-throttled rate (K=13/16, ~1.95 GHz; ~467us PE-busy vs
~512us in v8); the residual span beyond PE-busy comes from the
inter-core launch-skew barrier (~40-55us) that delays both AllGathers,
whose completion times vary run to run.

  A: local cq slice for the AllGather (xq, 192 cols covering chunks
     1-3 only), local kv slice (ckv 512 | k_r 64 roped | pad) -> stage2
     rows 0:640, plus PE transposes of the local ckv -> stage2 rows
     640:1152 (v in [t,d] layout). Two collectives total (ops have a
     ~25us fixed floor): AG1 (cq, 3MB out) then AG2v (kv+v, 4.5MB out).
  rec-c0: chunk 0 of cq is recomputed from x cols 0:512 (present on
     every core) -> B chunk 0 runs with no collective dependency and
     fills the barrier window; B chunks 1-3 consume the AG1 copy.
  C: flash-style causal attention per (512-chunk i4, head h), 128-key
     tiles, diagonal 512-blocks trimmed at 128-query granularity
     (descending order; the last psy matmul is the full-width r=0 tile
     so accumulation closes cleanly); 3-deep score/post software
     pipeline; ones-matmul + reciprocal + partition_broadcast ->
     normalize into SBUF-resident y^T.
  D: out^T = W_out_c^T @ y^T, tcc-outer so each PSUM drain overlaps
     the next tcc's matmuls; PSUM rotated over 5 tags -> outT f32.
Host sums the 8 partial outT (row-split TP gather) and transposes.

DMA rings (scalar/sync are fast HWDGE, gpsimd slow SWDGE): early feed
split scalar/sync in consumption order; AG triggers + early-key
readbacks on gpsimd (it resumes exactly when the collectives finish);
cqres readbacks on scalar (anything queued behind them needs the same
data); v + late-key readbacks on sync.
"""

import math
from contextlib import ExitStack

import numpy as np
import ml_dtypes

import concourse.bass as bass
import concourse.tile as tile
from concourse import bacc, mybir
from concourse.bass_utils import run_bass_kernel_spmd

F32 = mybir.dt.float32
F32R = mybir.dt.float32r
BF16 = mybir.dt.bfloat16
AF = mybir.ActivationFunctionType

T_FULL = 2048
E = 2048
KV = 512
QL = 1024
RH = 64
QKH = KV + RH     # 576
NH = 16
NCORES = 8
HPC = NH // NCORES
SCALE = 1.0 / math.sqrt(float(KV))

P = 128
NGQ = QL // P             # 8 c_q groups
NGKV = (KV + 2 * RH) // P  # 5 kv groups: ckv x4 + [kr|pad]
LATF = QL + KV + 2 * RH   # 1664 wlat columns: cq | ckv | kr | pad
TLOC = T_FULL // NCORES   # 256
SQ = (T_FULL - 512) // NCORES  # 192: cq-AG cols per core (chunks 1-3)


def build_kernel(T=T_FULL):
    assert T == T_FULL
    NT512 = T // 512
    NKT = T // P
    EK = E // P            # 16 contraction slices in phase A
    EKH = EK // 2
    QK = QL // P           # 8 contraction slices in phase B
    DK = HPC * KV // P     # 8 contraction slices in phase D

    nc = bacc.Bacc("TRN2", target_bir_lowering=False, debug=False,
                   num_devices=NCORES)

    xTf = nc.dram_tensor("xTf", [E, 512], BF16, kind="ExternalInput").ap()
    xTl = nc.dram_tensor("xTl", [E, TLOC], BF16, kind="ExternalInput").ap()
    xq = nc.dram_tensor("xq", [E, SQ], BF16, kind="ExternalInput").ap()
    wlat = nc.dram_tensor("wlat", [E, LATF], BF16, kind="ExternalInput").ap()
    wqd = nc.dram_tensor("wqd", [QL, HPC * QKH], BF16, kind="ExternalInput").ap()
    wout = nc.dram_tensor("wout", [HPC * KV, E], BF16, kind="ExternalInput").ap()
    cos2d = nc.dram_tensor("cos2", [P, T], BF16, kind="ExternalInput").ap()
    ssin2d = nc.dram_tensor("ssin2", [P, T], BF16, kind="ExternalInput").ap()
    klcosd = nc.dram_tensor("klcos", [RH, TLOC], BF16, kind="ExternalInput").ap()
    klsind = nc.dram_tensor("klsin", [RH, TLOC], BF16, kind="ExternalInput").ap()
    rotd = nc.dram_tensor("rot", [P, P], BF16, kind="ExternalInput").ap()
    identd = nc.dram_tensor("ident", [P, P], BF16, kind="ExternalInput").ap()
    masksd = nc.dram_tensor("masks", [P, 4 * 512], BF16,
                            kind="ExternalInput").ap()
    # aux f32 constants: col 0 = ones, cols 1:385 = zeros
    zauxd = nc.dram_tensor("zaux", [P, 385], F32, kind="ExternalInput").ap()
    outT = nc.dram_tensor("outT", [E, T], F32, kind="ExternalOutput").ap()

    xf_r = xTf.rearrange("(ko p) t -> p ko t", p=P)
    wl_r = wlat.rearrange("(ko p) m -> p ko m", p=P)

    with tile.TileContext(nc) as tc, ExitStack() as ctx:
        dram = ctx.enter_context(tc.tile_pool(name="dram", bufs=1, space="DRAM"))
        cst = ctx.enter_context(tc.tile_pool(name="cst", bufs=1))
        pp = ctx.enter_context(tc.tile_pool(name="pp", bufs=3, space="PSUM"))
        bw = ctx.enter_context(tc.tile_pool(name="bw", bufs=1))
        kvp1 = ctx.enter_context(tc.tile_pool(name="kvp1", bufs=1))
        cqp = ctx.enter_context(tc.tile_pool(name="cqp", bufs=2))
        bs = ctx.enter_context(tc.tile_pool(name="bs", bufs=2))
        # closed in order a0 (post-triggers), a1 (post rec-c0), a2 (post
        # kv-local) -- open in reverse so pool releases stay LIFO
        a2 = ExitStack()
        aw2 = a2.enter_context(tc.tile_pool(name="aw2", bufs=1))
        a1 = ExitStack()
        aw1 = a1.enter_context(tc.tile_pool(name="aw1", bufs=1))
        a0 = ExitStack()
        aw0 = a0.enter_context(tc.tile_pool(name="aw0", bufs=1))

        xloc = aw0.tile([P, EK, TLOC], BF16, tag="xloc")
        xq_sb = aw0.tile([P, EK, SQ], BF16, tag="xq")
        xl_r = xTl.rearrange("(ko p) t -> p ko t", p=P)
        wcq = [aw1.tile([P, EK, P], BF16, tag=f"wcq{g}", name=f"wcq{g}")
               for g in range(NGQ)]
        wkv = [aw2.tile([P, EK, P], BF16, tag=f"wkv{g}", name=f"wkv{g}")
               for g in range(NGKV)]
        xch0 = aw2.tile([P, EK, 512], BF16, tag="xch0")

        # ---- early feed, rings in consumption order ----
        # sync ring (stays light so the stage writes land early)
        nc.sync.dma_start(xq_sb[:, 0:EKH, :],
                          xq.rearrange("(ko p) t -> p ko t", p=P)[:, 0:EKH, :])
        nc.sync.dma_start(wcq[0][:, 0:EKH, :], wl_r[:, 0:EKH, 0:P])
        nc.sync.dma_start(wcq[0][:, EKH:EK, :], wl_r[:, EKH:EK, 0:P])
        nc.sync.dma_start(wcq[2][:], wl_r[:, :, 2 * P : 3 * P])
        nc.sync.dma_start(xloc[:, 0:EKH, :], xl_r[:, 0:EKH, :])
        nc.sync.dma_start(wcq[6][:], wl_r[:, :, 6 * P : 7 * P])
        nc.sync.dma_start(xch0[:, EKH:EK, :], xf_r[:, EKH:EK, :])
        # scalar ring
        nc.scalar.dma_start(xq_sb[:, EKH:EK, :],
                            xq.rearrange("(ko p) t -> p ko t", p=P)[:, EKH:EK, :])
        nc.scalar.dma_start(xloc[:, EKH:EK, :], xl_r[:, EKH:EK, :])
        for g in (1, 3, 5, 7):
            nc.scalar.dma_start(wcq[g][:], wl_r[:, :, g * P : (g + 1) * P])
        nc.scalar.dma_start(wkv[1][:], wl_r[:, :, (NGQ + 1) * P : (NGQ + 2) * P])
        nc.scalar.dma_start(wkv[3][:], wl_r[:, :, (NGQ + 3) * P : (NGQ + 4) * P])
        nc.scalar.dma_start(xch0[:, 0:EKH, :], xf_r[:, 0:EKH, :])
        # gpsimd ring (slow SWDGE): constants + remaining weights
        klcos = cst.tile([RH, TLOC], BF16, tag="klcos")
        nc.gpsimd.dma_start(klcos[:], klcosd[:])
        klsin = cst.tile([RH, TLOC], BF16, tag="klsin")
        nc.gpsimd.dma_start(klsin[:], klsind[:])
        rot = cst.tile([P, P], BF16, tag="rot")
        nc.gpsimd.dma_start(rot[:], rotd[:])
        ident = cst.tile([P, P], BF16, tag="ident")
        nc.gpsimd.dma_start(ident[:], identd[:])
        nc.gpsimd.dma_start(wkv[4][:], wl_r[:, :, (NGQ + 4) * P : LATF])
        nc.gpsimd.dma_start(wcq[4][:], wl_r[:, :, 4 * P : 5 * P])
        nc.gpsimd.dma_start(wkv[0][:], wl_r[:, :, QL : QL + P])
        nc.gpsimd.dma_start(wkv[2][:], wl_r[:, :, (NGQ + 2) * P : (NGQ + 3) * P])

        # collective stages: one cq AG + one merged kv+v AG
        stage1 = dram.tile([QL, SQ], BF16)
        ag1out = dram.tile([NCORES * QL, SQ], BF16, addr_space="Shared")
        ag1_r = ag1out[:].rearrange("(r g p) s -> p g r s", p=P, g=NGQ)
        NR2 = NGKV * P + 4 * P  # 1152: kv 0:640 | v-as-[512,256] 640:1152
        stage2 = dram.tile([NR2, TLOC], BF16)
        ag2out = dram.tile([NCORES * NR2, TLOC], BF16, addr_space="Shared")
        ag2_r = ag2out[:].rearrange("(r g p) s -> p g r s", p=P, g=NR2 // P)

        wqd_sb = bw.tile([P, QK, HPC * QKH], BF16, tag="wqd")
        wqd_r = wqd.rearrange("(ko p) m -> p ko m", p=P)
        cos2 = cst.tile([P, T], BF16, tag="cos2")
        ssin2 = cst.tile([P, T], BF16, tag="ssin2")

        # persistent residents
        ckvT = kvp1.tile([P, KV // P, T], BF16, tag="ckvT")
        # rows 0:64 and 64:128 both hold roped k_r so each head's score
        # matmul sees lhsT at the same base partition as its rhs qr slice
        krT2 = kvp1.tile([P, T], BF16, tag="krT2")
        qnT = [kvp1.tile([P, KV // P, T], BF16, tag=f"qnT{h}", name=f"qnT{h}")
               for h in range(HPC)]
        qrT2 = kvp1.tile([P, T], BF16, tag="qrT2")  # rows 0:64 h0, 64:128 h1
        v = kvp1.tile([P, NKT, KV], BF16, tag="v")

        # ============ Phase A: local slices -> 2 AllGathers =================
        with tc.tile_pool(name="ast", bufs=2) as ast, \
             tc.tile_pool(name="ptr", bufs=2, space="PSUM") as ptr:
            vst = ast.tile([P, 2, KV], BF16, tag="vst", name="vst", bufs=1)
            ORDER = [("q", 0), ("q", 1), ("q", 2), ("q", 3), ("k", 4),
                     ("q", 4), ("q", 5), ("k", 0), ("q", 6), ("q", 7),
                     ("k", 2), ("k", 1), ("k", 3)]
            for kind, g in ORDER:
                if kind == "q":
                    ps = pp.tile([P, SQ], F32, tag="mm", name="psAl")
                    for kc in range(EK):
                        nc.tensor.matmul(
                            ps[:], wcq[g][:, kc, :], xq_sb[:, kc, :],
                            start=(kc == 0), stop=(kc == EK - 1),
                        )
                    st = ast.tile([P, SQ], BF16, tag="astq", name="astq")
                    nc.vector.tensor_copy(st[:], ps[:])
                    nc.sync.dma_start(stage1[g * P : (g + 1) * P, :], st[:])
                    continue
                ps = pp.tile([P, TLOC], F32, tag="mm", name="psA")
                for kc in range(EK):
                    nc.tensor.matmul(
                        ps[:], wkv[g][:, kc, :], xloc[:, kc, :],
                        start=(kc == 0), stop=(kc == EK - 1),
                    )
                st = ast.tile([P, TLOC], BF16, tag="ast", name="ast")
                if g == NGKV - 1:
                    # rows 0:64 = k_r -> rope locally before staging
                    nc.gpsimd.memset(st[RH:P, :], 0.0)
                    kraw = ast.tile([RH, TLOC], BF16, tag="kraw", name="kraw")
                    nc.vector.tensor_copy(kraw[:], ps[0:RH, :])
                    pr = pp.tile([RH, TLOC], F32, tag="mm", name="prk")
                    nc.tensor.matmul(pr[:], rot[0:RH, 0:RH], kraw[:],
                                     start=True, stop=True)
                    nc.vector.tensor_mul(st[0:RH, :], kraw[:], klcos[:])
                    rotk = ast.tile([RH, TLOC], BF16, tag="rotk", name="rotk")
                    nc.vector.tensor_mul(rotk[:], pr[:], klsin[:])
                    nc.vector.tensor_add(st[0:RH, :], st[0:RH, :], rotk[:])
                else:
                    nc.vector.tensor_copy(st[:], ps[:])
                    for tt in range(2):
                        pt = ptr.tile([P, P], BF16, tag="tr", name="pt")
                        nc.tensor.transpose(
                            pt[:], st[:, tt * P : (tt + 1) * P], ident[:]
                        )
                        nc.scalar.copy(vst[:, tt, g * P : (g + 1) * P], pt[:])
                nc.sync.dma_start(stage2[g * P : (g + 1) * P, :], st[:])
            nc.sync.dma_start(
                stage2[NGKV * P : NR2, :].rearrange("(f p) c -> p f c", p=P),
                vst[:].rearrange("p two (dd c) -> p (two dd) c", dd=2),
            )
        # B-phase weights behind the stage writes (needed from ~70us)
        nc.scalar.dma_start(wqd_sb[:, 0 : QK // 2, :], wqd_r[:, 0 : QK // 2, :])
        nc.sync.dma_start(wqd_sb[:, QK // 2 : QK, :], wqd_r[:, QK // 2 : QK, :])
        nc.sync.dma_start(cos2[:], cos2d[:])
        nc.scalar.dma_start(ssin2[:], ssin2d[:])
        # triggers (gpsimd); emission == readiness == consumption order
        nc.gpsimd.collective_compute(
            "AllGather", mybir.AluOpType.bypass,
            replica_groups=[list(range(NCORES))],
            ins=[stage1.opt()], outs=[ag1out.opt()],
        )
        nc.gpsimd.collective_compute(
            "AllGather", mybir.AluOpType.bypass,
            replica_groups=[list(range(NCORES))],
            ins=[stage2.opt()], outs=[ag2out.opt()],
        )
        a0.close()

        # ============ rec-c0 + B chunk 0 (collective-free) ==================
        def b_chunk(tcc, cqres):
            qsl = slice(tcc * 512, (tcc + 1) * 512)
            for gm in range(9):
                ps = pp.tile([P, 512], F32, tag="mm", name="psB")
                for kc in range(QK):
                    nc.tensor.matmul(
                        ps[:], wqd_sb[:, kc, gm * P : (gm + 1) * P],
                        cqres[:, kc, :],
                        start=(kc == 0), stop=(kc == QK - 1),
                    )
                if gm == 0:
                    qraw = bs.tile([P, 512], BF16, tag="qraw", name="qraw")
                    nc.vector.tensor_copy(qraw[:], ps[:])
                    prq = pp.tile([P, 512], F32, tag="mm", name="prq")
                    nc.tensor.matmul(prq[:], rot[:], qraw[:],
                                     start=True, stop=True)
                    nc.vector.tensor_mul(qrT2[:, qsl], qraw[:], cos2[:, qsl])
                    rotq = bs.tile([P, 512], BF16, tag="rotq", name="rotq")
                    nc.vector.tensor_mul(rotq[:], prq[:], ssin2[:, qsl])
                    nc.vector.tensor_add(qrT2[:, qsl], qrT2[:, qsl], rotq[:])
                else:
                    h, dc = (gm - 1) // 4, (gm - 1) % 4
                    if gm % 2 == 1:
                        nc.vector.tensor_copy(qnT[h][:, dc, qsl], ps[:])
                    else:
                        nc.scalar.copy(qnT[h][:, dc, qsl], ps[:])

        def fetch_chunk(tcc, cqres):
            # chunk tcc covers global t [512*tcc, 512*(tcc+1)); the AG holds
            # t = 512 + SQ*r + s
            c0 = 512 * tcc - 512
            r = c0 // SQ
            col = 0
            while col < 512:
                s0 = c0 + col - r * SQ
                n = min(SQ - s0, 512 - col)
                nc.scalar.dma_start(
                    cqres[:, :, col : col + n], ag1_r[:, :, r, s0 : s0 + n]
                )
                col += n
                r += 1

        # chunk 0: recompute from x cols 0:512 (kc order h2-first: the
        # sync-ring xch0 half lands before the scalar-ring half)
        cqres0 = cqp.tile([P, QK, 512], BF16, tag="cqres", name="cqres0")
        KCO = list(range(EKH, EK)) + list(range(EKH))
        for g in range(NGQ):
            ps = pp.tile([P, 512], F32, tag="mm", name="psAq")
            for j, kc in enumerate(KCO):
                nc.tensor.matmul(
                    ps[:], wcq[g][:, kc, :], xch0[:, kc, :],
                    start=(j == 0), stop=(j == EK - 1),
                )
            if g % 2 == 0:
                nc.vector.tensor_copy(cqres0[:, g, :], ps[:])
            else:
                nc.scalar.copy(cqres0[:, g, :], ps[:])
        a1.close()
        b_chunk(0, cqres0)
        a2.close()

        # late constants + C/D residents (after the A pools free)
        kvp2 = ctx.enter_context(tc.tile_pool(name="kvp2", bufs=1))
        masksb = kvp2.tile([P, 4 * 512], BF16, tag="masksb")
        nc.scalar.dma_start(masksb[:], masksd[:])
        zaux = kvp2.tile([P, 385], F32, tag="zaux")
        nc.scalar.dma_start(zaux[:], zauxd[:])
        ones_col = kvp2.tile([P, 1], F32R, tag="ones")
        nc.vector.tensor_copy(ones_col[:], zaux[:, 0:1])
        zeros0 = zaux[:, 1:385]
        masks = [masksb[:, r * 512 : (r + 1) * 512] for r in range(4)]
        yT = kvp2.tile([P, DK, T], BF16, tag="yT")
        accD = kvp2.tile([P, 512], F32R, tag="accD")
        denb = kvp2.tile([P, 512], F32, tag="denb")
        wop = ctx.enter_context(tc.tile_pool(name="wop", bufs=1))
        wout_sb = wop.tile([P, DK, E], BF16, tag="wout")

        # AG2v readbacks consolidated into few large DMAs (the many small
        # r-major DMAs polluted shared DMAHW completion lanes, creating
        # false waits that stalled B's tail). kv on gpsimd (which resumes
        # exactly when the collectives finish), v on sync.
        for dc in range(KV // P):
            nc.gpsimd.dma_start(
                ckvT[:, dc, :].rearrange("p (r s) -> p r s", r=NCORES),
                ag2_r[:, dc, :, :],
            )
        nc.gpsimd.dma_start(
            krT2[0:RH, :].rearrange("p (r s) -> p r s", r=NCORES),
            ag2_r[0:RH, 4, :, :],
        )
        nc.gpsimd.dma_start(krT2[RH:P, :], krT2[0:RH, :])
        for r in range(NCORES):
            nc.sync.dma_start(
                v[:, 2 * r : 2 * r + 2, :].rearrange(
                    "p two (dd c) -> p (two dd) c", dd=2),
                ag2_r[:, NGKV : NGKV + 4, r, :],
            )

        # ============ Phase C machinery =====================================
        cctx = ExitStack()
        cs = cctx.enter_context(tc.tile_pool(name="cs", bufs=2))
        ppy = cctx.enter_context(tc.tile_pool(name="ppy", bufs=1, space="PSUM"))
        pden = cctx.enter_context(tc.tile_pool(name="pden", bufs=1,
                                               space="PSUM"))

        def attn_block(i4, h):
            # (ksl, off, mask_r): off = query-column offset into the 512-
            # chunk (free-dim trimming of diagonal blocks, descending so
            # the last psy matmul is full-width and closes accumulation)
            tiles = [(slice(j * P, (j + 1) * P), 0, None)
                     for j in range(4 * i4)]
            base = 4 * i4
            for r in (3, 2, 1, 0):
                tiles.append(
                    (slice((base + r) * P, (base + r + 1) * P),
                     r * P if r else 0, r)
                )
            nj = len(tiles)
            psy = [ppy.tile([P, 512], F32, tag=f"y{dc}", name=f"psy{dc}")
                   for dc in range(KV // P)]
            q0 = i4 * 512

            def scores(idx):
                ksl, off, _ = tiles[idx]
                qs = slice(q0 + off, q0 + 512)
                ps = pp.tile([P, 512], F32, tag="mm", name=f"psS{idx % 3}")
                for dc in range(KV // P):
                    nc.tensor.matmul(
                        ps[:, off:512], ckvT[:, dc, ksl], qnT[h][:, dc, qs],
                        start=(dc == 0), stop=False,
                    )
                nc.tensor.matmul(
                    ps[:, off:512], krT2[h * RH : (h + 1) * RH, ksl],
                    qrT2[h * RH : (h + 1) * RH, qs],
                    start=False, stop=True,
                )
                return ps

            def post(idx, ps):
                ksl, off, mr = tiles[idx]
                se = cs.tile([P, 512], BF16, tag="se", bufs=6, name="se")
                nc.scalar.activation(se[:, off:512], ps[:, off:512], AF.Exp)
                if mr is not None:
                    nc.vector.tensor_mul(
                        se[:, off:512], se[:, off:512], masks[mr][:, off:512]
                    )
                if idx == 0:
                    if off:
                        nc.vector.tensor_copy(accD[:, 0:off], zeros0[:, 0:off])
                    nc.vector.tensor_copy(accD[:, off:512], se[:, off:512])
                else:
                    nc.vector.tensor_add(
                        accD[:, off:512], accD[:, off:512], se[:, off:512]
                    )
                first = (idx == 0)
                last = (idx == nj - 1)
                for dc in range(KV // P):
                    nc.tensor.matmul(
                        psy[dc][:, off:512],
                        v[:, ksl.start // P, dc * P : (dc + 1) * P],
                        se[:, off:512],
                        start=first, stop=last,
                    )

            # three-deep score/post software pipeline (pp has 3 bufs)
            depth = min(3, nj)
            ring = [scores(i) for i in range(depth)]
            for idx in range(depth, nj):
                ring.append(scores(idx))
                post(idx - depth, ring.pop(0))
            while ring:
                post(nj - len(ring), ring.pop(0))

            psden = pden.tile([1, 512], F32, tag="den", name="psden")
            nc.tensor.matmul(psden[:], ones_col[:], accD[:],
                             start=True, stop=True)
            deninv = cs.tile([1, 512], F32, tag="deninv", name="deninv")
            nc.vector.reciprocal_approx_fast(out=deninv[:], in_=psden[:])
            nc.gpsimd.partition_broadcast(denb[:], deninv[:])
            qsl = slice(i4 * 512, (i4 + 1) * 512)
            for dc in range(KV // P):
                nc.vector.tensor_mul(
                    yT[:, h * (KV // P) + dc, qsl], psy[dc][:], denb[:]
                )

        # ============ B chunks 1-3 (cqres from AG1) ========================
        for tcc in range(1, NT512):
            cqres = cqp.tile([P, QK, 512], BF16, tag="cqres",
                             name=f"cqres{tcc}")
            fetch_chunk(tcc, cqres)
            b_chunk(tcc, cqres)

        # wout rides scalar from here (clear of the collectives' window)
        nc.scalar.dma_start(
            wout_sb[:], wout.rearrange("(ko p) e -> p ko e", p=P)
        )

        # ============ Phase C: attention over gathered keys ================
        for i4 in range(NT512):
            for h in range(HPC):
                attn_block(i4, h)
        cctx.close()

        # ============ Phase D: out^T = W_out_c^T @ y^T ======================
        with ExitStack() as dctx:
            dst = dctx.enter_context(tc.tile_pool(name="dst", bufs=3))
            ppd = dctx.enter_context(tc.tile_pool(name="ppd", bufs=1,
                                                  space="PSUM"))
            for mc in range(E // P):
                for tcc in range(NT512):
                    psD = ppd.tile([P, 512], F32,
                                   tag=f"d{(mc * NT512 + tcc) % 5}",
                                   name=f"psD{tcc}")
                    for kc in range(DK):
                        nc.tensor.matmul(
                            psD[:],
                            wout_sb[:, kc, mc * P : (mc + 1) * P],
                            yT[:, kc, tcc * 512 : (tcc + 1) * 512],
                            start=(kc == 0), stop=(kc == DK - 1),
                        )
                    ost = dst.tile([P, 512], F32, tag=f"ost{tcc % 2}",
                                   name="ost")
                    if tcc % 2 == 0:
                        nc.vector.tensor_copy(ost[:], psD[:])
                    else:
                        nc.scalar.copy(ost[:], psD[:])
                    (nc.scalar if tcc % 2 else nc.sync).dma_start(
                        outT[mc * P : (mc + 1) * P,
                             tcc * 512 : (tcc + 1) * 512],
                        ost[:],
                    )

    nc.compile()
    return nc


_NC_CACHE = {}


def _get_nc(T=T_FULL):
    if T not in _NC_CACHE:
        _NC_CACHE[T] = build_kernel(T)
    return _NC_CACHE[T]


def make_in_maps(x, cos, sin, W_qkv, W_qdec, W_out):
    bf = ml_dtypes.bfloat16
    x = np.asarray(x)
    xT = np.ascontiguousarray(x[0].T).astype(bf)           # [E, T]
    W_qkv = np.asarray(W_qkv).astype(np.float32)
    W_qdec = np.asarray(W_qdec).astype(np.float32)
    W_out = np.asarray(W_out).astype(np.float32)
    cos = np.asarray(cos).astype(np.float32)
    sin = np.asarray(sin).astype(np.float32)

    # Wlat columns: cq 1024 | ckv 512 | kr 64 | pad 64  (replicated)
    wlat = np.zeros((E, LATF), np.float32)
    wlat[:, 0:QL] = W_qkv[:, QKH:]
    wlat[:, QL : QL + KV] = W_qkv[:, 0:KV]
    wlat[:, QL + KV : QL + KV + RH] = W_qkv[:, KV : KV + RH]
    wlat = wlat.astype(bf)

    cosT = cos.T.copy()                                     # [64, T]
    ssinT = sin.T.copy()
    ssinT[0 : RH // 2] *= -1.0
    cos2 = np.vstack([cosT, cosT]).astype(bf)               # [128, T]
    ssin2 = np.vstack([ssinT, ssinT]).astype(bf)

    # RT0[x, y] = 1 iff x == (y+32) % 64; rot = blockdiag(RT0, RT0)
    rt0 = np.zeros((RH, RH), np.float32)
    rt0[(np.arange(RH) + RH // 2) % RH, np.arange(RH)] = 1.0
    rot = np.zeros((P, P), np.float32)
    rot[0:RH, 0:RH] = rt0
    rot[RH:P, RH:P] = rt0
    rot = rot.astype(bf)
    ident = np.eye(P, dtype=np.float32).astype(bf)
    # masks[r][p, q] = 1 iff q >= p + 128 r (stair mask for diagonal tiles)
    masks = np.zeros((P, 4, 512), np.float32)
    qq = np.arange(512)[None, :]
    pp_ = np.arange(P)[:, None]
    for r in range(4):
        masks[:, r, :] = (qq >= pp_ + P * r)
    masks = masks.reshape(P, 4 * 512).astype(bf)
    zaux = np.zeros((P, 385), np.float32)
    zaux[:, 0] = 1.0

    Wq = W_qdec * SCALE
    in_maps = []
    for c in range(NCORES):
        h0, h1 = 2 * c, 2 * c + 1
        # wqd cols: [qr_h0 64 | qr_h1 64 | qn_h0 512 | qn_h1 512]
        wqd_c = np.concatenate(
            [
                Wq[:, h0 * QKH + KV : (h0 + 1) * QKH],
                Wq[:, h1 * QKH + KV : (h1 + 1) * QKH],
                Wq[:, h0 * QKH : h0 * QKH + KV],
                Wq[:, h1 * QKH : h1 * QKH + KV],
            ],
            axis=1,
        ).astype(bf)
        tsl = slice(c * TLOC, (c + 1) * TLOC)
        in_maps.append({
            "xTf": np.ascontiguousarray(xT[:, 0:512]),
            "xq": np.ascontiguousarray(xT[:, 512 + c * SQ : 512 + (c + 1) * SQ]),
            "xTl": np.ascontiguousarray(xT[:, tsl]),
            "wlat": wlat,
            "wqd": wqd_c,
            "wout": W_out[c * HPC * KV : (c + 1) * HPC * KV].astype(bf),
            "cos2": cos2,
            "ssin2": ssin2,
            "klcos": np.ascontiguousarray(cos2[0:RH, tsl]),
            "klsin": np.ascontiguousarray(ssin2[0:RH, tsl]),
            "rot": rot,
            "ident": ident,
            "masks": masks,
            "zaux": zaux,
        })
    return in_maps


def kernel(x, cos, sin, W_qkv, W_qdec, W_out, _trace=False, _tmpdir=None):
    T = np.asarray(x).shape[1]
    nc = _get_nc(T)
    in_maps = make_in_maps(x, cos, sin, W_qkv, W_qdec, W_out)
    res = run_bass_kernel_spmd(
        nc, in_maps, core_ids=list(range(NCORES)),
        trace=_trace, tmpdir=_tmpdir,
    )
    out = np.zeros((E, T), np.float32)
    for r in res.results:
        out += r["outT"]
    kernel.last_results = res
    return np.ascontiguousarray(out.T)[None].astype(np.float32)


# revision 100
# speedup vs baseline: 1.0158x; 1.0158x over previous
"""MLA-style causal self-attention on 8 Trainium2 NeuronCores.

v24: tensor-parallel over heads (2/core). The kernel is PE-cycle-bound
at the GPIO-throttled rate (K=13/16, ~1.95 GHz; ~467us PE-busy vs
~512us in v8); the residual span beyond PE-busy comes from the
inter-core launch-skew barrier (~40-55us) that delays both AllGathers,
whose completion times vary run to run.

  A: local cq slice for the AllGather (xq, 192 cols covering chunks
     1-3 only), local kv slice (ckv 512 | k_r 64 roped | pad) -> stage2
     rows 0:640, plus PE transposes of the local ckv -> stage2 rows
     640:1152 (v in [t,d] layout). Two collectives total (ops have a
     ~25us fixed floor): AG1 (cq, 3MB out) then AG2v (kv+v, 4.5MB out).
  rec-c0: chunk 0 of cq is recomputed from x cols 0:512 (present on
     every core) -> B chunk 0 runs with no collective dependency and
     fills the barrier window; B chunks 1-3 consume the AG1 copy.
  C: flash-style causal attention per (512-chunk i4, head h), 128-key
     tiles, diagonal 512-blocks trimmed at 128-query granularity
     (descending order; the last psy matmul is the full-width r=0 tile
     so accumulation closes cleanly); 3-deep score/post software
     pipeline; ones-matmul + reciprocal + partition_broadcast ->
     normalize into SBUF-resident y^T.
  D: out^T = W_out_c^T @ y^T, tcc-outer so each PSUM drain overlaps
     the next tcc's matmuls; PSUM rotated over 5 tags -> outT f32.
Host sums the 8 partial outT (row-split TP gather) and transposes.

DMA rings (scalar/sync are fast HWDGE, gpsimd slow SWDGE): early feed
split scalar/sync in consumption order; AG triggers + early-key
readbacks on gpsimd (it resumes exactly when the collectives finish);
cqres readbacks on scalar (anything queued behind them needs the same
data); v + late-key readbacks on sync.
"""

import math
from contextlib import ExitStack

import numpy as np
import ml_dtypes

import concourse.bass as bass
import concourse.tile as tile
from concourse import bacc, mybir
from concourse.bass_utils import run_bass_kernel_spmd

F32 = mybir.dt.float32
F32R = mybir.dt.float32r
BF16 = mybir.dt.bfloat16
AF = mybir.ActivationFunctionType

T_FULL = 2048
E = 2048
KV = 512
QL = 1024
RH = 64
QKH = KV + RH     # 576
NH = 16
NCORES = 8
HPC = NH // NCORES
SCALE = 1.0 / math.sqrt(float(KV))

P = 128
NGQ = QL // P             # 8 c_q groups
NGKV = (KV + 2 * RH) // P  # 5 kv groups: ckv x4 + [kr|pad]
LATF = QL + KV + 2 * RH   # 1664 wlat columns: cq | ckv | kr | pad
TLOC = T_FULL // NCORES   # 256
SQ = (T_FULL - 512) // NCORES  # 192: cq-AG cols per core (chunks 1-3)


def build_kernel(T=T_FULL):
    assert T == T_FULL
    NT512 = T // 512
    NKT = T // P
    EK = E // P            # 16 contraction slices in phase A
    EKH = EK // 2
    QK = QL // P           # 8 contraction slices in phase B
    DK = HPC * KV // P     # 8 contraction slices in phase D

    nc = bacc.Bacc("TRN2", target_bir_lowering=False, debug=False,
                   num_devices=NCORES)

    xTf = nc.dram_tensor("xTf", [E, 512], BF16, kind="ExternalInput").ap()
    xTl = nc.dram_tensor("xTl", [E, TLOC], BF16, kind="ExternalInput").ap()
    xq = nc.dram_tensor("xq", [E, SQ], BF16, kind="ExternalInput").ap()
    wlat = nc.dram_tensor("wlat", [E, LATF], BF16, kind="ExternalInput").ap()
    wqd = nc.dram_tensor("wqd", [QL, HPC * QKH], BF16, kind="ExternalInput").ap()
    wout = nc.dram_tensor("wout", [HPC * KV, E], BF16, kind="ExternalInput").ap()
    cos2d = nc.dram_tensor("cos2", [P, T], BF16, kind="ExternalInput").ap()
    ssin2d = nc.dram_tensor("ssin2", [P, T], BF16, kind="ExternalInput").ap()
    klcosd = nc.dram_tensor("klcos", [RH, TLOC], BF16, kind="ExternalInput").ap()
    klsind = nc.dram_tensor("klsin", [RH, TLOC], BF16, kind="ExternalInput").ap()
    rotd = nc.dram_tensor("rot", [P, P], BF16, kind="ExternalInput").ap()
    identd = nc.dram_tensor("ident", [P, P], BF16, kind="ExternalInput").ap()
    masksd = nc.dram_tensor("masks", [P, 4 * 512], BF16,
                            kind="ExternalInput").ap()
    # aux f32 constants: col 0 = ones, cols 1:385 = zeros
    zauxd = nc.dram_tensor("zaux", [P, 385], F32, kind="ExternalInput").ap()
    outT = nc.dram_tensor("outT", [E, T], F32, kind="ExternalOutput").ap()

    xf_r = xTf.rearrange("(ko p) t -> p ko t", p=P)
    wl_r = wlat.rearrange("(ko p) m -> p ko m", p=P)

    with tile.TileContext(nc) as tc, ExitStack() as ctx:
        dram = ctx.enter_context(tc.tile_pool(name="dram", bufs=1, space="DRAM"))
        cst = ctx.enter_context(tc.tile_pool(name="cst", bufs=1))
        pp = ctx.enter_context(tc.tile_pool(name="pp", bufs=3, space="PSUM"))
        bw = ctx.enter_context(tc.tile_pool(name="bw", bufs=1))
        kvp1 = ctx.enter_context(tc.tile_pool(name="kvp1", bufs=1))
        cqp = ctx.enter_context(tc.tile_pool(name="cqp", bufs=2))
        bs = ctx.enter_context(tc.tile_pool(name="bs", bufs=2))
        # closed in order a0 (post-triggers), a1 (post rec-c0), a2 (post
        # kv-local) -- open in reverse so pool releases stay LIFO
        a2 = ExitStack()
        aw2 = a2.enter_context(tc.tile_pool(name="aw2", bufs=1))
        a1 = ExitStack()
        aw1 = a1.enter_context(tc.tile_pool(name="aw1", bufs=1))
        a0 = ExitStack()
        aw0 = a0.enter_context(tc.tile_pool(name="aw0", bufs=1))

        xloc = aw0.tile([P, EK, TLOC], BF16, tag="xloc")
        xq_sb = aw0.tile([P, EK, SQ], BF16, tag="xq")
        xl_r = xTl.rearrange("(ko p) t -> p ko t", p=P)
        wcq = [aw1.tile([P, EK, P], BF16, tag=f"wcq{g}", name=f"wcq{g}")
               for g in range(NGQ)]
        wkv = [aw2.tile([P, EK, P], BF16, tag=f"wkv{g}", name=f"wkv{g}")
               for g in range(NGKV)]
        xch0 = aw2.tile([P, EK, 512], BF16, tag="xch0")

        # ---- early feed, rings in consumption order ----
        # sync ring (stays light so the stage writes land early)
        nc.sync.dma_start(xq_sb[:, 0:EKH, :],
                          xq.rearrange("(ko p) t -> p ko t", p=P)[:, 0:EKH, :])
        nc.sync.dma_start(wcq[0][:, 0:EKH, :], wl_r[:, 0:EKH, 0:P])
        nc.sync.dma_start(wcq[0][:, EKH:EK, :], wl_r[:, EKH:EK, 0:P])
        nc.sync.dma_start(wcq[2][:], wl_r[:, :, 2 * P : 3 * P])
        nc.sync.dma_start(xloc[:, 0:EKH, :], xl_r[:, 0:EKH, :])
        nc.sync.dma_start(wcq[6][:], wl_r[:, :, 6 * P : 7 * P])
        nc.sync.dma_start(xch0[:, EKH:EK, :], xf_r[:, EKH:EK, :])
        # scalar ring
        nc.scalar.dma_start(xq_sb[:, EKH:EK, :],
                            xq.rearrange("(ko p) t -> p ko t", p=P)[:, EKH:EK, :])
        nc.scalar.dma_start(xloc[:, EKH:EK, :], xl_r[:, EKH:EK, :])
        for g in (1, 3, 5, 7):
            nc.scalar.dma_start(wcq[g][:], wl_r[:, :, g * P : (g + 1) * P])
        nc.scalar.dma_start(wkv[1][:], wl_r[:, :, (NGQ + 1) * P : (NGQ + 2) * P])
        nc.scalar.dma_start(wkv[3][:], wl_r[:, :, (NGQ + 3) * P : (NGQ + 4) * P])
        nc.scalar.dma_start(xch0[:, 0:EKH, :], xf_r[:, 0:EKH, :])
        # gpsimd ring (slow SWDGE): constants + remaining weights
        klcos = cst.tile([RH, TLOC], BF16, tag="klcos")
        nc.gpsimd.dma_start(klcos[:], klcosd[:])
        klsin = cst.tile([RH, TLOC], BF16, tag="klsin")
        nc.gpsimd.dma_start(klsin[:], klsind[:])
        rot = cst.tile([P, P], BF16, tag="rot")
        nc.gpsimd.dma_start(rot[:], rotd[:])
        ident = cst.tile([P, P], BF16, tag="ident")
        nc.gpsimd.dma_start(ident[:], identd[:])
        nc.gpsimd.dma_start(wkv[4][:], wl_r[:, :, (NGQ + 4) * P : LATF])
        nc.gpsimd.dma_start(wcq[4][:], wl_r[:, :, 4 * P : 5 * P])
        nc.gpsimd.dma_start(wkv[0][:], wl_r[:, :, QL : QL + P])
        nc.gpsimd.dma_start(wkv[2][:], wl_r[:, :, (NGQ + 2) * P : (NGQ + 3) * P])

        # collective stages: one cq AG + one merged kv+v AG
        stage1 = dram.tile([QL, SQ], BF16)
        ag1out = dram.tile([NCORES * QL, SQ], BF16, addr_space="Shared")
        ag1_r = ag1out[:].rearrange("(r g p) s -> p g r s", p=P, g=NGQ)
        NR2 = NGKV * P + 4 * P  # 1152: kv 0:640 | v-as-[512,256] 640:1152
        stage2 = dram.tile([NR2, TLOC], BF16)
        ag2out = dram.tile([NCORES * NR2, TLOC], BF16, addr_space="Shared")
        ag2_r = ag2out[:].rearrange("(r g p) s -> p g r s", p=P, g=NR2 // P)

        wqd_sb = bw.tile([P, QK, HPC * QKH], BF16, tag="wqd")
        wqd_r = wqd.rearrange("(ko p) m -> p ko m", p=P)
        cos2 = cst.tile([P, T], BF16, tag="cos2")
        ssin2 = cst.tile([P, T], BF16, tag="ssin2")

        # persistent residents
        ckvT = kvp1.tile([P, KV // P, T], BF16, tag="ckvT")
        # rows 0:64 and 64:128 both hold roped k_r so each head's score
        # matmul sees lhsT at the same base partition as its rhs qr slice
        krT2 = kvp1.tile([P, T], BF16, tag="krT2")
        qnT = [kvp1.tile([P, KV // P, T], BF16, tag=f"qnT{h}", name=f"qnT{h}")
               for h in range(HPC)]
        qrT2 = kvp1.tile([P, T], BF16, tag="qrT2")  # rows 0:64 h0, 64:128 h1
        v = kvp1.tile([P, NKT, KV], BF16, tag="v")

        # ============ Phase A: local slices -> 2 AllGathers =================
        with tc.tile_pool(name="ast", bufs=2) as ast, \
             tc.tile_pool(name="ptr", bufs=2, space="PSUM") as ptr:
            vst = ast.tile([P, 2, KV], BF16, tag="vst", name="vst", bufs=1)
            # all cq groups first (their weights ride the fast rings and
            # arrive ~2us apart), kv after (wkv0/2/4 trail on the slow
            # gpsimd ring); also puts stage1's writes strictly before
            # stage2's last write, cleaning the cc readiness order
            ORDER = [("q", 0), ("q", 1), ("q", 2), ("q", 3), ("k", 4),
                     ("q", 4), ("q", 5), ("q", 6), ("q", 7), ("k", 0),
                     ("k", 1), ("k", 2), ("k", 3)]
            for kind, g in ORDER:
                if kind == "q":
                    ps = pp.tile([P, SQ], F32, tag="mm", name="psAl")
                    for kc in range(EK):
                        nc.tensor.matmul(
                            ps[:], wcq[g][:, kc, :], xq_sb[:, kc, :],
                            start=(kc == 0), stop=(kc == EK - 1),
                        )
                    st = ast.tile([P, SQ], BF16, tag="astq", name="astq")
                    nc.vector.tensor_copy(st[:], ps[:])
                    nc.sync.dma_start(stage1[g * P : (g + 1) * P, :], st[:])
                    continue
                ps = pp.tile([P, TLOC], F32, tag="mm", name="psA")
                for kc in range(EK):
                    nc.tensor.matmul(
                        ps[:], wkv[g][:, kc, :], xloc[:, kc, :],
                        start=(kc == 0), stop=(kc == EK - 1),
                    )
                st = ast.tile([P, TLOC], BF16, tag="ast", name="ast")
                if g == NGKV - 1:
                    # rows 0:64 = k_r -> rope locally before staging
                    nc.gpsimd.memset(st[RH:P, :], 0.0)
                    kraw = ast.tile([RH, TLOC], BF16, tag="kraw", name="kraw")
                    nc.vector.tensor_copy(kraw[:], ps[0:RH, :])
                    pr = pp.tile([RH, TLOC], F32, tag="mm", name="prk")
                    nc.tensor.matmul(pr[:], rot[0:RH, 0:RH], kraw[:],
                                     start=True, stop=True)
                    nc.vector.tensor_mul(st[0:RH, :], kraw[:], klcos[:])
                    rotk = ast.tile([RH, TLOC], BF16, tag="rotk", name="rotk")
                    nc.vector.tensor_mul(rotk[:], pr[:], klsin[:])
                    nc.vector.tensor_add(st[0:RH, :], st[0:RH, :], rotk[:])
                else:
                    nc.vector.tensor_copy(st[:], ps[:])
                    for tt in range(2):
                        pt = ptr.tile([P, P], BF16, tag="tr", name="pt")
                        nc.tensor.transpose(
                            pt[:], st[:, tt * P : (tt + 1) * P], ident[:]
                        )
                        nc.scalar.copy(vst[:, tt, g * P : (g + 1) * P], pt[:])
                nc.sync.dma_start(stage2[g * P : (g + 1) * P, :], st[:])
            nc.sync.dma_start(
                stage2[NGKV * P : NR2, :].rearrange("(f p) c -> p f c", p=P),
                vst[:].rearrange("p two (dd c) -> p (two dd) c", dd=2),
            )
        # B-phase weights behind the stage writes (needed from ~70us)
        nc.scalar.dma_start(wqd_sb[:, 0 : QK // 2, :], wqd_r[:, 0 : QK // 2, :])
        nc.sync.dma_start(wqd_sb[:, QK // 2 : QK, :], wqd_r[:, QK // 2 : QK, :])
        nc.sync.dma_start(cos2[:], cos2d[:])
        nc.scalar.dma_start(ssin2[:], ssin2d[:])
        # triggers (gpsimd); emission == readiness == consumption order
        nc.gpsimd.collective_compute(
            "AllGather", mybir.AluOpType.bypass,
            replica_groups=[list(range(NCORES))],
            ins=[stage1.opt()], outs=[ag1out.opt()],
        )
        nc.gpsimd.collective_compute(
            "AllGather", mybir.AluOpType.bypass,
            replica_groups=[list(range(NCORES))],
            ins=[stage2.opt()], outs=[ag2out.opt()],
        )
        a0.close()

        # ============ rec-c0 + B chunk 0 (collective-free) ==================
        def b_chunk(tcc, cqres):
            qsl = slice(tcc * 512, (tcc + 1) * 512)
            for gm in range(9):
                ps = pp.tile([P, 512], F32, tag="mm", name="psB")
                for kc in range(QK):
                    nc.tensor.matmul(
                        ps[:], wqd_sb[:, kc, gm * P : (gm + 1) * P],
                        cqres[:, kc, :],
                        start=(kc == 0), stop=(kc == QK - 1),
                    )
                if gm == 0:
                    qraw = bs.tile([P, 512], BF16, tag="qraw", name="qraw")
                    nc.vector.tensor_copy(qraw[:], ps[:])
                    prq = pp.tile([P, 512], F32, tag="mm", name="prq")
                    nc.tensor.matmul(prq[:], rot[:], qraw[:],
                                     start=True, stop=True)
                    nc.vector.tensor_mul(qrT2[:, qsl], qraw[:], cos2[:, qsl])
                    rotq = bs.tile([P, 512], BF16, tag="rotq", name="rotq")
                    nc.vector.tensor_mul(rotq[:], prq[:], ssin2[:, qsl])
                    nc.vector.tensor_add(qrT2[:, qsl], qrT2[:, qsl], rotq[:])
                else:
                    h, dc = (gm - 1) // 4, (gm - 1) % 4
                    if gm % 2 == 1:
                        nc.vector.tensor_copy(qnT[h][:, dc, qsl], ps[:])
                    else:
                        nc.scalar.copy(qnT[h][:, dc, qsl], ps[:])

        def fetch_chunk(tcc, cqres):
            # chunk tcc covers global t [512*tcc, 512*(tcc+1)); the AG holds
            # t = 512 + SQ*r + s
            c0 = 512 * tcc - 512
            r = c0 // SQ
            col = 0
            while col < 512:
                s0 = c0 + col - r * SQ
                n = min(SQ - s0, 512 - col)
                nc.scalar.dma_start(
                    cqres[:, :, col : col + n], ag1_r[:, :, r, s0 : s0 + n]
                )
                col += n
                r += 1

        # chunk 0: recompute from x cols 0:512 (kc order h2-first: the
        # sync-ring xch0 half lands before the scalar-ring half)
        cqres0 = cqp.tile([P, QK, 512], BF16, tag="cqres", name="cqres0")
        KCO = list(range(EKH, EK)) + list(range(EKH))
        for g in range(NGQ):
            ps = pp.tile([P, 512], F32, tag="mm", name="psAq")
            for j, kc in enumerate(KCO):
                nc.tensor.matmul(
                    ps[:], wcq[g][:, kc, :], xch0[:, kc, :],
                    start=(j == 0), stop=(j == EK - 1),
                )
            if g % 2 == 0:
                nc.vector.tensor_copy(cqres0[:, g, :], ps[:])
            else:
                nc.scalar.copy(cqres0[:, g, :], ps[:])
        a1.close()
        b_chunk(0, cqres0)
        a2.close()

        # late constants + C/D residents (after the A pools free)
        kvp2 = ctx.enter_context(tc.tile_pool(name="kvp2", bufs=1))
        masksb = kvp2.tile([P, 4 * 512], BF16, tag="masksb")
        nc.scalar.dma_start(masksb[:], masksd[:])
        zaux = kvp2.tile([P, 385], F32, tag="zaux")
        nc.scalar.dma_start(zaux[:], zauxd[:])
        ones_col = kvp2.tile([P, 1], F32R, tag="ones")
        nc.vector.tensor_copy(ones_col[:], zaux[:, 0:1])
        zeros0 = zaux[:, 1:385]
        masks = [masksb[:, r * 512 : (r + 1) * 512] for r in range(4)]
        yT = kvp2.tile([P, DK, T], BF16, tag="yT")
        accD = kvp2.tile([P, 512], F32R, tag="accD")
        denb = kvp2.tile([P, 512], F32, tag="denb")
        wop = ctx.enter_context(tc.tile_pool(name="wop", bufs=1))
        wout_sb = wop.tile([P, DK, E], BF16, tag="wout")

        # AG2v readbacks consolidated into few large DMAs (the many small
        # r-major DMAs polluted shared DMAHW completion lanes, creating
        # false waits that stalled B's tail). kv on gpsimd (which resumes
        # exactly when the collectives finish), v on sync.
        for dc in range(KV // P):
            nc.gpsimd.dma_start(
                ckvT[:, dc, :].rearrange("p (r s) -> p r s", r=NCORES),
                ag2_r[:, dc, :, :],
            )
        nc.gpsimd.dma_start(
            krT2[0:RH, :].rearrange("p (r s) -> p r s", r=NCORES),
            ag2_r[0:RH, 4, :, :],
        )
        nc.gpsimd.dma_start(krT2[RH:P, :], krT2[0:RH, :])
        for r in range(NCORES):
            nc.sync.dma_start(
                v[:, 2 * r : 2 * r + 2, :].rearrange(
                    "p two (dd c) -> p (two dd) c", dd=2),
                ag2_r[:, NGKV : NGKV + 4, r, :],
            )

        # ============ Phase C machinery =====================================
        cctx = ExitStack()
        cs = cctx.enter_context(tc.tile_pool(name="cs", bufs=2))
        ppy = cctx.enter_context(tc.tile_pool(name="ppy", bufs=1, space="PSUM"))
        pden = cctx.enter_context(tc.tile_pool(name="pden", bufs=1,
                                               space="PSUM"))

        def attn_block(i4, h):
            # (ksl, off, mask_r): off = query-column offset into the 512-
            # chunk (free-dim trimming of diagonal blocks, descending so
            # the last psy matmul is full-width and closes accumulation)
            tiles = [(slice(j * P, (j + 1) * P), 0, None)
                     for j in range(4 * i4)]
            base = 4 * i4
            for r in (3, 2, 1, 0):
                tiles.append(
                    (slice((base + r) * P, (base + r + 1) * P),
                     r * P if r else 0, r)
                )
            nj = len(tiles)
            psy = [ppy.tile([P, 512], F32, tag=f"y{dc}", name=f"psy{dc}")
                   for dc in range(KV // P)]
            q0 = i4 * 512

            def scores(idx):
                ksl, off, _ = tiles[idx]
                qs = slice(q0 + off, q0 + 512)
                ps = pp.tile([P, 512], F32, tag="mm", name=f"psS{idx % 3}")
                for dc in range(KV // P):
                    nc.tensor.matmul(
                        ps[:, off:512], ckvT[:, dc, ksl], qnT[h][:, dc, qs],
                        start=(dc == 0), stop=False,
                    )
                nc.tensor.matmul(
                    ps[:, off:512], krT2[h * RH : (h + 1) * RH, ksl],
                    qrT2[h * RH : (h + 1) * RH, qs],
                    start=False, stop=True,
                )
                return ps

            def post(idx, ps):
                ksl, off, mr = tiles[idx]
                se = cs.tile([P, 512], BF16, tag="se", bufs=6, name="se")
                nc.scalar.activation(se[:, off:512], ps[:, off:512], AF.Exp)
                if mr is not None:
                    nc.vector.tensor_mul(
                        se[:, off:512], se[:, off:512], masks[mr][:, off:512]
                    )
                if idx == 0:
                    if off:
                        nc.vector.tensor_copy(accD[:, 0:off], zeros0[:, 0:off])
                    nc.vector.tensor_copy(accD[:, off:512], se[:, off:512])
                else:
                    nc.vector.tensor_add(
                        accD[:, off:512], accD[:, off:512], se[:, off:512]
                    )
                first = (idx == 0)
                last = (idx == nj - 1)
                for dc in range(KV // P):
                    nc.tensor.matmul(
                        psy[dc][:, off:512],
                        v[:, ksl.start // P, dc * P : (dc + 1) * P],
                        se[:, off:512],
                        start=first, stop=last,
                    )

            # three-deep score/post software pipeline (pp has 3 bufs)
            depth = min(3, nj)
            ring = [scores(i) for i in range(depth)]
            for idx in range(depth, nj):
                ring.append(scores(idx))
                post(idx - depth, ring.pop(0))
            while ring:
                post(nj - len(ring), ring.pop(0))

            psden = pden.tile([1, 512], F32, tag="den", name="psden")
            nc.tensor.matmul(psden[:], ones_col[:], accD[:],
                             start=True, stop=True)
            deninv = cs.tile([1, 512], F32, tag="deninv", name="deninv")
            nc.vector.reciprocal_approx_fast(out=deninv[:], in_=psden[:])
            nc.gpsimd.partition_broadcast(denb[:], deninv[:])
            qsl = slice(i4 * 512, (i4 + 1) * 512)
            for dc in range(KV // P):
                nc.vector.tensor_mul(
                    yT[:, h * (KV // P) + dc, qsl], psy[dc][:], denb[:]
                )

        # ============ B chunks 1-3 (cqres from AG1) ========================
        for tcc in range(1, NT512):
            cqres = cqp.tile([P, QK, 512], BF16, tag="cqres",
                             name=f"cqres{tcc}")
            fetch_chunk(tcc, cqres)
            b_chunk(tcc, cqres)

        # wout rides scalar from here (clear of the collectives' window)
        nc.scalar.dma_start(
            wout_sb[:], wout.rearrange("(ko p) e -> p ko e", p=P)
        )

        # ============ Phase C: attention over gathered keys ================
        for i4 in range(NT512):
            for h in range(HPC):
                attn_block(i4, h)
        cctx.close()

        # ============ Phase D: out^T = W_out_c^T @ y^T ======================
        with ExitStack() as dctx:
            dst = dctx.enter_context(tc.tile_pool(name="dst", bufs=3))
            ppd = dctx.enter_context(tc.tile_pool(name="ppd", bufs=1,
                                                  space="PSUM"))
            for mc in range(E // P):
                for tcc in range(NT512):
                    psD = ppd.tile([P, 512], F32,
                                   tag=f"d{(mc * NT512 + tcc) % 5}",
                                   name=f"psD{tcc}")
                    for kc in range(DK):
                        nc.tensor.matmul(
                            psD[:],
                            wout_sb[:, kc, mc * P : (mc + 1) * P],
                            yT[:, kc, tcc * 512 : (tcc + 1) * 512],
                            start=(kc == 0), stop=(kc == DK - 1),
                        )
                    ost = dst.tile([P, 512], F32, tag=f"ost{tcc % 2}",
                                   name="ost")
                    if tcc % 2 == 0:
                        nc.vector.tensor_copy(ost[:], psD[:])
                    else:
                        nc.scalar.copy(ost[:], psD[:])
                    (nc.scalar if tcc % 2 else nc.sync).dma_start(
                        outT[mc * P : (mc + 1) * P,
                             tcc * 512 : (tcc + 1) * 512],
                        ost[:],
                    )

    nc.compile()
    return nc


_NC_CACHE = {}


def _get_nc(T=T_FULL):
    if T not in _NC_CACHE:
        _NC_CACHE[T] = build_kernel(T)
    return _NC_CACHE[T]


def make_in_maps(x, cos, sin, W_qkv, W_qdec, W_out):
    bf = ml_dtypes.bfloat16
    x = np.asarray(x)
    xT = np.ascontiguousarray(x[0].T).astype(bf)           # [E, T]
    W_qkv = np.asarray(W_qkv).astype(np.float32)
    W_qdec = np.asarray(W_qdec).astype(np.float32)
    W_out = np.asarray(W_out).astype(np.float32)
    cos = np.asarray(cos).astype(np.float32)
    sin = np.asarray(sin).astype(np.float32)

    # Wlat columns: cq 1024 | ckv 512 | kr 64 | pad 64  (replicated)
    wlat = np.zeros((E, LATF), np.float32)
    wlat[:, 0:QL] = W_qkv[:, QKH:]
    wlat[:, QL : QL + KV] = W_qkv[:, 0:KV]
    wlat[:, QL + KV : QL + KV + RH] = W_qkv[:, KV : KV + RH]
    wlat = wlat.astype(bf)

    cosT = cos.T.copy()                                     # [64, T]
    ssinT = sin.T.copy()
    ssinT[0 : RH // 2] *= -1.0
    cos2 = np.vstack([cosT, cosT]).astype(bf)               # [128, T]
    ssin2 = np.vstack([ssinT, ssinT]).astype(bf)

    # RT0[x, y] = 1 iff x == (y+32) % 64; rot = blockdiag(RT0, RT0)
    rt0 = np.zeros((RH, RH), np.float32)
    rt0[(np.arange(RH) + RH // 2) % RH, np.arange(RH)] = 1.0
    rot = np.zeros((P, P), np.float32)
    rot[0:RH, 0:RH] = rt0
    rot[RH:P, RH:P] = rt0
    rot = rot.astype(bf)
    ident = np.eye(P, dtype=np.float32).astype(bf)
    # masks[r][p, q] = 1 iff q >= p + 128 r (stair mask for diagonal tiles)
    masks = np.zeros((P, 4, 512), np.float32)
    qq = np.arange(512)[None, :]
    pp_ = np.arange(P)[:, None]
    for r in range(4):
        masks[:, r, :] = (qq >= pp_ + P * r)
    masks = masks.reshape(P, 4 * 512).astype(bf)
    zaux = np.zeros((P, 385), np.float32)
    zaux[:, 0] = 1.0

    Wq = W_qdec * SCALE
    in_maps = []
    for c in range(NCORES):
        h0, h1 = 2 * c, 2 * c + 1
        # wqd cols: [qr_h0 64 | qr_h1 64 | qn_h0 512 | qn_h1 512]
        wqd_c = np.concatenate(
            [
                Wq[:, h0 * QKH + KV : (h0 + 1) * QKH],
                Wq[:, h1 * QKH + KV : (h1 + 1) * QKH],
                Wq[:, h0 * QKH : h0 * QKH + KV],
                Wq[:, h1 * QKH : h1 * QKH + KV],
            ],
            axis=1,
        ).astype(bf)
        tsl = slice(c * TLOC, (c + 1) * TLOC)
        in_maps.append({
            "xTf": np.ascontiguousarray(xT[:, 0:512]),
            "xq": np.ascontiguousarray(xT[:, 512 + c * SQ : 512 + (c + 1) * SQ]),
            "xTl": np.ascontiguousarray(xT[:, tsl]),
            "wlat": wlat,
            "wqd": wqd_c,
            "wout": W_out[c * HPC * KV : (c + 1) * HPC * KV].astype(bf),
            "cos2": cos2,
            "ssin2": ssin2,
            "klcos": np.ascontiguousarray(cos2[0:RH, tsl]),
            "klsin": np.ascontiguousarray(ssin2[0:RH, tsl]),
            "rot": rot,
            "ident": ident,
            "masks": masks,
            "zaux": zaux,
        })
    return in_maps


def kernel(x, cos, sin, W_qkv, W_qdec, W_out, _trace=False, _tmpdir=None):
    T = np.asarray(x).shape[1]
    nc = _get_nc(T)
    in_maps = make_in_maps(x, cos, sin, W_qkv, W_qdec, W_out)
    res = run_bass_kernel_spmd(
        nc, in_maps, core_ids=list(range(NCORES)),
        trace=_trace, tmpdir=_tmpdir,
    )
    out = np.zeros((E, T), np.float32)
    for r in res.results:
        out += r["outT"]
    kernel.last_results = res
    return np.ascontiguousarray(out.T)[None].astype(np.float32)
